# revision 15
# baseline (speedup 1.0000x reference)
"""Trainium2 Bass kernel for a cross-attention transformer layer.

Reference computation (per batch b):
    Q = query @ Wq.T + bq ; K = key @ Wk.T + bk ; V = value @ Wv.T + bv
    scores = QK^T/sqrt(d_k) per head, masked, softmax
    out = LayerNorm(softmax(scores) V @ Wo.T + bo + query)

Sharding: 8 cores = 4 batches x 2 query-halves. Each core computes the
full layer for its (batch, 1024-query-row) shard; K/V projections are
recomputed per half (no collectives needed). Output shards concatenate.

Device-side layout is the "transposed world": activations live as
[d_model, seq] (d on partitions) so projections, scores, attn*V and the
output projection chain into each other with no transposes:
  QT[d,q] = WqT.T @ queryT ;  KT[d,k] = WkT.T @ keyT   (head pairs share
     a 128-partition block: head h at partitions 64*(h%2), chunk h//2)
  V[k,d]  = valueT.T @ WvT                     (natural [k,d] layout)
  scoresT[k,q] = KT_h.T @ QT_h   row-packed: the two heads of a pair run
     concurrently on PE row strips (0,0)/(64,0) (64-dim contractions)
  PT[k,q] = exp(scoresT/8 + maskbias[k])  - one ACT op per [128,1024]
     PSUM tile; the mask rides in the per-partition bias AP
  ctx     = V_h.T @ PT  col-packed: pair heads write partitions 0:64 /
     64:128 of one PSUM tile via tile_position (0,0)/(0,64), fp32
     accumulation over the 16 k tiles
  sums    = ones.T @ PT  (M=1 matmuls col-packed to partition rows
     {0,32,64,96}; softmax denominators, same fp32 accumulation)
  Z[q,o]  = sum_do ctx[:,do].T @ WoT[:,do] + residual; LayerNorm over o.
"""

import sys

if "/opt/trn_rl_repo" not in sys.path:
    sys.path.insert(0, "/opt/trn_rl_repo")

import numpy as np
import ml_dtypes

import concourse.bacc as bacc
import concourse.mybir as mybir
import concourse.tile as tile
from concourse import bass_utils

F32 = mybir.dt.float32
BF16 = mybir.dt.bfloat16
AF = mybir.ActivationFunctionType
ALU = mybir.AluOpType

D_MODEL = 512
N_HEADS = 8
D_K = 64
SQ = 1024          # query rows per core
SK = 2048          # key rows per core
N_CORES = 8
P = 128
NEG = -1.0e9

_NC_CACHE: dict = {}


def _build(qkv_bias: bool, ln_affine: bool):
    """Build the per-core NEFF. All 8 cores run this same program."""
    nc = bacc.Bacc("TRN2", target_bir_lowering=False, debug=False,
                   enable_asserts=False, num_devices=N_CORES)

    d = lambda name, shape, dt: nc.dram_tensor(name, shape, dt, kind="ExternalInput").ap()
    qT = d("qT", [D_MODEL, SQ], BF16)
    kT = d("kT", [D_MODEL, SK], BF16)
    vT = d("vT", [D_MODEL, SK], BF16)
    qres = d("qres", [SQ, D_MODEL], F32)        # query rows + bo (residual)
    wqT = d("wqT", [D_MODEL, D_MODEL], BF16)    # Wq^T  [c_in, d_out]
    wkT = d("wkT", [D_MODEL, D_MODEL], BF16)
    wvT = d("wvT", [D_MODEL, D_MODEL], BF16)
    woT = d("woT", [D_MODEL, D_MODEL], BF16)
    maskbias = d("maskbias", [SK], F32)         # 0 / -1e9 per key
    if qkv_bias:
        bq = d("bq", [D_MODEL], F32)
        bk = d("bk", [D_MODEL], F32)
        bv = d("bv", [D_MODEL], F32)
    if ln_affine:
        gamma = d("gamma", [D_MODEL], F32)
        beta = d("beta", [D_MODEL], F32)
    out = nc.dram_tensor("out", [SQ, D_MODEL], F32, kind="ExternalOutput").ap()

    CO = D_MODEL // P   # 4 outer chunks of the model dim
    KT_TILES = SK // P  # 16 key tiles
    NPAIR = N_HEADS // 2

    with tile.TileContext(nc) as tc:
        with (
            tc.tile_pool(name="singles", bufs=1) as singles,
            tc.tile_pool(name="inbuf", bufs=1) as inbuf,
            tc.tile_pool(name="pt", bufs=6) as ptp,
            tc.tile_pool(name="small", bufs=4) as small,
            tc.tile_pool(name="stream", bufs=3) as stream,
            # PSUM budget (8 banks): sc 2x[128,1024]=4, ctx [128,2,512]=2,
            # sums [128,2,512]=2.  proj/Z/rb matmuls share the "sc" slots.
            tc.tile_pool(name="ps_sc", bufs=2, space="PSUM") as ps_sc,
            tc.tile_pool(name="ps_ctx", bufs=1, space="PSUM") as ps_ctx,
            tc.tile_pool(name="ps_sums", bufs=1, space="PSUM") as ps_sums,
        ):
            # ---- load weights + small params -------------------------------
            w_sb = {}
            for name, ap in (("wq", wqT), ("wk", wkT), ("wv", wvT), ("wo", woT)):
                t = singles.tile([P, CO, D_MODEL], BF16, tag=f"w_{name}")
                nc.sync.dma_start(t[:], ap.rearrange("(co ci) o -> ci co o", ci=P))
                w_sb[name] = t

            mb_sb = singles.tile([P, KT_TILES], F32, tag="mb")
            nc.sync.dma_start(mb_sb[:], maskbias.rearrange("(kt ki) -> ki kt", ki=P))

            if qkv_bias:
                bq_sb = singles.tile([P, CO], F32, tag="bq")
                nc.sync.dma_start(bq_sb[:], bq.rearrange("(co ci) -> ci co", ci=P))
                bk_sb = singles.tile([P, CO], F32, tag="bk")
                nc.sync.dma_start(bk_sb[:], bk.rearrange("(co ci) -> ci co", ci=P))
                bv_bc = singles.tile([P, D_MODEL], F32, tag="bv")
                nc.sync.dma_start(bv_bc[:], bv.to_broadcast((P, D_MODEL)))
            if ln_affine:
                gamma_bc = singles.tile([P, D_MODEL], F32, tag="gamma")
                nc.sync.dma_start(gamma_bc[:], gamma.to_broadcast((P, D_MODEL)))
                beta_bc = singles.tile([P, D_MODEL], F32, tag="beta")
                nc.sync.dma_start(beta_bc[:], beta.to_broadcast((P, D_MODEL)))
            eps_sb = singles.tile([P, 1], F32, tag="eps")
            nc.gpsimd.memset(eps_sb[:], 1e-5)
            ones128 = singles.tile([P, 1], BF16, tag="ones128")
            nc.gpsimd.memset(ones128[:], 1.0)
            ones1 = singles.tile([1, D_K], BF16, tag="ones1")
            nc.gpsimd.memset(ones1[:], 1.0)

            # ---- load activations (transposed layouts) ---------------------
            qT_sb = inbuf.tile([P, CO, SQ], BF16, tag="qT")
            nc.sync.dma_start(qT_sb[:], qT.rearrange("(co ci) q -> ci co q", ci=P))
            kT_sb = inbuf.tile([P, CO, SK], BF16, tag="kT")
            nc.sync.dma_start(kT_sb[:], kT.rearrange("(co ci) k -> ci co k", ci=P))
            vT_sb = inbuf.tile([P, CO, SK], BF16, tag="vT")
            nc.sync.dma_start(vT_sb[:], vT.rearrange("(co ci) k -> ci co k", ci=P))

            # ---- projections (psums ride in the "sc" slots) ----------------
            QT_sb = singles.tile([P, CO, SQ], BF16, tag="QT")
            for do in range(CO):
                for q0 in range(0, SQ, 512):
                    psq = ps_sc.tile([P, 512], F32, tag="sc")
                    for ci in range(CO):
                        nc.tensor.matmul(
                            psq[:], w_sb["wq"][:, ci, do * P:(do + 1) * P],
                            qT_sb[:, ci, q0:q0 + 512],
                            start=(ci == 0), stop=(ci == CO - 1))
                    dst = QT_sb[:, do, q0:q0 + 512]
                    if qkv_bias:
                        nc.vector.tensor_scalar_add(dst, psq[:], bq_sb[:, do:do + 1])
                    else:
                        nc.vector.tensor_copy(dst, psq[:])

            KT_sb = singles.tile([P, CO, SK], BF16, tag="KT")
            for do in range(CO):
                for k0 in range(0, SK, 512):
                    psk = ps_sc.tile([P, 512], F32, tag="sc")
                    for ci in range(CO):
                        nc.tensor.matmul(
                            psk[:], w_sb["wk"][:, ci, do * P:(do + 1) * P],
                            kT_sb[:, ci, k0:k0 + 512],
                            start=(ci == 0), stop=(ci == CO - 1))
                    dst = KT_sb[:, do, k0:k0 + 512]
                    if qkv_bias:
                        nc.vector.tensor_scalar_add(dst, psk[:], bk_sb[:, do:do + 1])
                    else:
                        nc.vector.tensor_copy(dst, psk[:])

            V_sb = singles.tile([P, KT_TILES, N_HEADS, D_K], BF16, tag="V")
            for st in range(KT_TILES):
                psv = ps_sc.tile([P, 512], F32, tag="sc")
                for ci in range(CO):
                    nc.tensor.matmul(
                        psv[:], vT_sb[:, ci, st * P:(st + 1) * P],
                        w_sb["wv"][:, ci, :],
                        start=(ci == 0), stop=(ci == CO - 1))
                dst = V_sb[:, st, :, :]
                src = psv[:].rearrange("p (h e) -> p h e", h=N_HEADS)
                if qkv_bias:
                    nc.vector.tensor_tensor(
                        dst, src,
                        bv_bc[:].rearrange("p (h e) -> p h e", h=N_HEADS),
                        ALU.add)
                else:
                    nc.vector.tensor_copy(dst, src)

            # ---- attention: head pairs, full 1024-q tiles ------------------
            # ctx_sb[d, do, q] pair-major (matches O-proj lhsT layout)
            ctx_sb = singles.tile([P, CO, SQ], BF16, tag="ctx")
            # softmax denominators, row j = (head, q-chunk) = 2h + c
            sums_sb = singles.tile([2 * N_HEADS, 512], F32, tag="sums")
            sums_stage = {}  # (group) -> strided psum tile with 4 sums rows

            for pair in range(NPAIR):
                hA, hB = 2 * pair, 2 * pair + 1
                do = pair
                grp, gi = divmod(pair, 2)
                if gi == 0:
                    sums_stage[grp] = ps_sums.tile([P, 2, 512], F32, tag="sums", name=f"sums_g{grp}")
                sm = sums_stage[grp]
                rA, rB = 64 * gi, 64 * gi + 32
                ctx_ps = ps_ctx.tile([P, 2, 512], F32, tag="ctx")
                for kt in range(KT_TILES):
                    k0 = kt * P
                    sA = ps_sc.tile([P, 1024], F32, tag="sc")
                    sB = ps_sc.tile([P, 1024], F32, tag="sc")
                    for c in range(2):
                        q0 = c * 512
                        nc.tensor.matmul(
                            sA[:, q0:q0 + 512],
                            KT_sb[0:D_K, do, k0:k0 + P],
                            QT_sb[0:D_K, do, q0:q0 + 512],
                            start=True, stop=True)
                        nc.tensor.matmul(
                            sB[:, q0:q0 + 512],
                            KT_sb[D_K:P, do, k0:k0 + P],
                            QT_sb[D_K:P, do, q0:q0 + 512],
                            start=True, stop=True)
                    ptA = ptp.tile([P, 1024], BF16, tag="pt")
                    nc.scalar.activation(ptA[:], sA[:], AF.Exp,
                                         bias=mb_sb[:, kt:kt + 1], scale=0.125)
                    ptB = ptp.tile([P, 1024], BF16, tag="pt")
                    nc.scalar.activation(ptB[:], sB[:], AF.Exp,
                                         bias=mb_sb[:, kt:kt + 1], scale=0.125)
                    first, last = kt == 0, kt == KT_TILES - 1
                    for c in range(2):
                        q0 = c * 512
                        nc.tensor.matmul(
                            ctx_ps[0:D_K, c, :], V_sb[:, kt, hA, :],
                            ptA[:, q0:q0 + 512], start=first, stop=last,
                            tile_position=(0, 0), skip_group_check=True)
                        nc.tensor.matmul(
                            ctx_ps[D_K:P, c, :], V_sb[:, kt, hB, :],
                            ptB[:, q0:q0 + 512], start=first, stop=last,
                            tile_position=(0, D_K), skip_group_check=True)
                        nc.tensor.matmul(
                            sm[rA:rA + 1, c, :], ones128[:],
                            ptA[:, q0:q0 + 512], start=first, stop=last,
                            tile_position=(0, rA), skip_group_check=True)
                        nc.tensor.matmul(
                            sm[rB:rB + 1, c, :], ones128[:],
                            ptB[:, q0:q0 + 512], start=first, stop=last,
                            tile_position=(0, rB), skip_group_check=True)
                # unnormalized context -> SBUF bf16 (normalized later)
                nc.vector.tensor_copy(
                    ctx_sb[:, do, :].rearrange("p (c q) -> p c q", c=2),
                    ctx_ps[:])
                if gi == 1:
                    # both pairs of this group done: move the 4 sums rows
                    # (partitions 0,32,64,96) out of PSUM, then to their
                    # (head, chunk) slots in sums_sb via DMA partition moves
                    sst = small.tile([P, 2, 512], F32, tag="sst")
                    for r in (0, 32, 64, 96):
                        nc.vector.tensor_copy(sst[r:r + 1, :, :],
                                              sm[r:r + 1, :, :])
                    for i, h in enumerate(range(2 * (pair - 1), 2 * pair + 2)):
                        r = 32 * i
                        for c in range(2):
                            nc.sync.dma_start(sums_sb[2 * h + c:2 * h + c + 1, :],
                                              sst[r:r + 1, c, :])

            # ---- normalize context ----------------------------------------
            recip = singles.tile([2 * N_HEADS, 512], F32, tag="recip")
            nc.vector.reciprocal(recip[:], sums_sb[:])
            recip_bf = singles.tile([2 * N_HEADS, 512], BF16, tag="recipbf")
            nc.vector.tensor_copy(recip_bf[:], recip[:])
            # stage rows into partition 0's free dim, then broadcast each
            # across 64 PSUM partitions via 1-row PE outer products
            stage = singles.tile([1, 2 * N_HEADS, 512], BF16, tag="stg")
            for j in range(2 * N_HEADS):
                nc.sync.dma_start(stage[:, j, :], recip_bf[j:j + 1, :])
            for pair in range(NPAIR):
                hA, hB = 2 * pair, 2 * pair + 1
                for c in range(2):
                    rb = ps_sc.tile([P, 512], F32, tag="sc")
                    nc.tensor.matmul(rb[0:D_K, :], ones1[:],
                                     stage[:, 2 * hA + c, :],
                                     start=True, stop=True,
                                     tile_position=(0, 0),
                                     skip_group_check=True)
                    nc.tensor.matmul(rb[D_K:P, :], ones1[:],
                                     stage[:, 2 * hB + c, :],
                                     start=True, stop=True,
                                     tile_position=(0, D_K),
                                     skip_group_check=True)
                    cslice = ctx_sb[:, pair, c * 512:(c + 1) * 512]
                    nc.vector.tensor_tensor(cslice, cslice, rb[:], ALU.mult)

            # ---- output projection + residual + layernorm ------------------
            for qc in range(SQ // P):
                zps = ps_sc.tile([P, D_MODEL], F32, tag="sc")
                for do in range(CO):
                    nc.tensor.matmul(
                        zps[:], ctx_sb[:, do, qc * P:(qc + 1) * P],
                        w_sb["wo"][:, do, :],
                        start=(do == 0), stop=(do == CO - 1))
                qr = stream.tile([P, D_MODEL], F32, tag="qr")
                nc.sync.dma_start(qr[:], qres[qc * P:(qc + 1) * P, :])
                z = stream.tile([P, D_MODEL], F32, tag="z")
                nc.vector.tensor_tensor(z[:], zps[:], qr[:], ALU.add)
                stats = small.tile([P, 6], F32, tag="stats")
                nc.vector.bn_stats(stats[:], z[:])
                mv = small.tile([P, 2], F32, tag="mv")
                nc.vector.bn_aggr(mv[:], stats[:])
                istd = small.tile([P, 1], F32, tag="istd")
                nc.scalar.activation(istd[:], mv[:, 1:2], AF.Sqrt,
                                     bias=eps_sb[:], scale=1.0)
                nc.vector.reciprocal(istd[:], istd[:])
                zo = stream.tile([P, D_MODEL], F32, tag="zo")
                nc.vector.tensor_scalar(zo[:], z[:], mv[:, 0:1], istd[:],
                                        ALU.subtract, ALU.mult)
                if ln_affine:
                    nc.vector.tensor_tensor(zo[:], zo[:], gamma_bc[:], ALU.mult)
                    nc.vector.tensor_tensor(zo[:], zo[:], beta_bc[:], ALU.add)
                nc.sync.dma_start(out[qc * P:(qc + 1) * P, :], zo[:])

    nc.compile()
    return nc


def _get_nc(qkv_bias: bool, ln_affine: bool):
    key = (qkv_bias, ln_affine)
    if key not in _NC_CACHE:
        _NC_CACHE[key] = _build(*key)
    return _NC_CACHE[key]


def prepare(query, key, value, key_mask, Wq, bq, Wk, bk, Wv, bv, Wo, bo,
            ln_gamma, ln_beta):
    """Host-side prep: returns (nc, in_maps) for the 8 cores."""
    query = np.asarray(query, dtype=np.float32)
    key = np.asarray(key, dtype=np.float32)
    value = np.asarray(value, dtype=np.float32)
    key_mask = np.asarray(key_mask)
    Wq = np.asarray(Wq, dtype=np.float32)
    Wk = np.asarray(Wk, dtype=np.float32)
    Wv = np.asarray(Wv, dtype=np.float32)
    Wo = np.asarray(Wo, dtype=np.float32)
    bq = np.asarray(bq, dtype=np.float32)
    bk = np.asarray(bk, dtype=np.float32)
    bv = np.asarray(bv, dtype=np.float32)
    bo = np.asarray(bo, dtype=np.float32)
    ln_gamma = np.asarray(ln_gamma, dtype=np.float32)
    ln_beta = np.asarray(ln_beta, dtype=np.float32)

    B, sq_full, dm = query.shape
    assert (B, sq_full, dm) == (4, 2048, 512), query.shape

    qkv_bias = bool(bq.any() or bk.any() or bv.any())
    ln_affine = bool((ln_gamma != 1.0).any() or ln_beta.any())
    nc = _get_nc(qkv_bias, ln_affine)

    bf = ml_dtypes.bfloat16
    wqT = np.ascontiguousarray(Wq.T).astype(bf)
    wkT = np.ascontiguousarray(Wk.T).astype(bf)
    wvT = np.ascontiguousarray(Wv.T).astype(bf)
    woT = np.ascontiguousarray(Wo.T).astype(bf)
    maskbias = np.where(key_mask, np.float32(0.0), np.float32(NEG))
    qres_full = query + bo[None, None, :]

    in_maps = []
    for core in range(N_CORES):
        b, half = divmod(core, 2)
        rows = slice(half * SQ, (half + 1) * SQ)
        m = {
            "qT": np.ascontiguousarray(query[b, rows].T).astype(bf),
            "kT": np.ascontiguousarray(key[b].T).astype(bf),
            "vT": np.ascontiguousarray(value[b].T).astype(bf),
            "qres": np.ascontiguousarray(qres_full[b, rows]),
            "wqT": wqT, "wkT": wkT, "wvT": wvT, "woT": woT,
            "maskbias": np.ascontiguousarray(maskbias[b]),
        }
        if qkv_bias:
            m["bq"], m["bk"], m["bv"] = bq, bk, bv
        if ln_affine:
            m["gamma"], m["beta"] = ln_gamma, ln_beta
        in_maps.append(m)
    return nc, in_maps


def kernel(**inputs):
    nc, in_maps = prepare(**inputs)
    B, sq_full, dm = 4, 2048, 512

    res = bass_utils.run_bass_kernel_spmd(nc, in_maps,
                                          core_ids=list(range(N_CORES)))
    out = np.empty((B, sq_full, dm), dtype=np.float32)
    for core in range(N_CORES):
        b, half = divmod(core, 2)
        out[b, half * SQ:(half + 1) * SQ] = res.results[core]["out"]
    return out


# revision 17
# speedup vs baseline: 71.7297x; 71.7297x over previous
"""Trainium2 Bass kernel for a cross-attention transformer layer.

Reference computation (per batch b):
    Q = query @ Wq.T + bq ; K = key @ Wk.T + bk ; V = value @ Wv.T + bv
    scores = QK^T/sqrt(d_k) per head, masked, softmax
    out = LayerNorm(softmax(scores) V @ Wo.T + bo + query)

Sharding: 8 cores = 4 batches x 2 query-halves. Each core computes the
full layer for its (batch, 1024-query-row) shard; K/V projections are
recomputed per half (no collectives needed). Output shards concatenate.

Device-side layout is the "transposed world": activations live as
[d_model, seq] (d on partitions) so projections, scores, attn*V and the
output projection chain into each other with no transposes:
  QT[d,q] = WqT.T @ queryT ;  KT[d,k] = WkT.T @ keyT   (head pairs share
     a 128-partition block: head h at partitions 64*(h%2), chunk h//2)
  V[k,d]  = valueT.T @ WvT                     (natural [k,d] layout)
  scoresT[k,q] = KT_h.T @ QT_h   row-packed: the two heads of a pair run
     concurrently on PE row strips (0,0)/(64,0) (64-dim contractions)
  PT[k,q] = exp(scoresT/8 + maskbias[k])  - one ACT op per [128,1024]
     PSUM tile; the mask rides in the per-partition bias AP
  ctx     = V_h.T @ PT  col-packed: pair heads write partitions 0:64 /
     64:128 of one PSUM tile via tile_position (0,0)/(0,64), fp32
     accumulation over the 16 k tiles
  sums    = ones.T @ PT  (M=1 matmuls col-packed to partition rows
     {0,32,64,96}; softmax denominators, same fp32 accumulation)
  Z[q,o]  = sum_do ctx[:,do].T @ WoT[:,do] + residual; LayerNorm over o.
"""

import sys

if "/opt/trn_rl_repo" not in sys.path:
    sys.path.insert(0, "/opt/trn_rl_repo")

import numpy as np
import ml_dtypes

import concourse.bacc as bacc
import concourse.mybir as mybir
import concourse.tile as tile
from concourse import bass_utils

F32 = mybir.dt.float32
BF16 = mybir.dt.bfloat16
AF = mybir.ActivationFunctionType
ALU = mybir.AluOpType

D_MODEL = 512
N_HEADS = 8
D_K = 64
SQ = 1024          # query rows per core
SK = 2048          # key rows per core
N_CORES = 8
P = 128
NEG = -1.0e9

_NC_CACHE: dict = {}


def _build(qkv_bias: bool, ln_affine: bool):
    """Build the per-core NEFF. All 8 cores run this same program."""
    nc = bacc.Bacc("TRN2", target_bir_lowering=False, debug=False,
                   enable_asserts=False, num_devices=N_CORES)

    d = lambda name, shape, dt: nc.dram_tensor(name, shape, dt, kind="ExternalInput").ap()
    qT = d("qT", [D_MODEL, SQ], BF16)
    kT = d("kT", [D_MODEL, SK], BF16)
    vT = d("vT", [D_MODEL, SK], BF16)
    qres = d("qres", [SQ, D_MODEL], F32)        # query rows + bo (residual)
    wqT = d("wqT", [D_MODEL, D_MODEL], BF16)    # Wq^T  [c_in, d_out]
    wkT = d("wkT", [D_MODEL, D_MODEL], BF16)
    wvT = d("wvT", [D_MODEL, D_MODEL], BF16)
    woT = d("woT", [D_MODEL, D_MODEL], BF16)
    maskbias = d("maskbias", [SK], F32)         # 0 / -1e9 per key
    if qkv_bias:
        bq = d("bq", [D_MODEL], F32)
        bk = d("bk", [D_MODEL], F32)
        bv = d("bv", [D_MODEL], F32)
    if ln_affine:
        gamma = d("gamma", [D_MODEL], F32)
        beta = d("beta", [D_MODEL], F32)
    out = nc.dram_tensor("out", [SQ, D_MODEL], F32, kind="ExternalOutput").ap()

    CO = D_MODEL // P   # 4 outer chunks of the model dim
    KT_TILES = SK // P  # 16 key tiles
    NPAIR = N_HEADS // 2

    with tile.TileContext(nc) as tc:
        with (
            tc.tile_pool(name="singles", bufs=1) as singles,
            tc.tile_pool(name="inbuf", bufs=1) as inbuf,
            tc.tile_pool(name="pt", bufs=8) as ptp,
            tc.tile_pool(name="small", bufs=4) as small,
            tc.tile_pool(name="stream", bufs=3) as stream,
            # PSUM budget (8 banks): sc 2x[128,1024]=4, ctx [128,2,512]=2,
            # sums [128,2,512]=2.  proj/Z/rb matmuls share the "sc" slots.
            tc.tile_pool(name="ps_sc", bufs=2, space="PSUM") as ps_sc,
            tc.tile_pool(name="ps_ctx", bufs=1, space="PSUM") as ps_ctx,
            tc.tile_pool(name="ps_sums", bufs=1, space="PSUM") as ps_sums,
        ):
            # ---- load weights + small params -------------------------------
            w_sb = {}
            for name, ap in (("wq", wqT), ("wk", wkT), ("wv", wvT), ("wo", woT)):
                t = singles.tile([P, CO, D_MODEL], BF16, tag=f"w_{name}")
                nc.sync.dma_start(t[:], ap.rearrange("(co ci) o -> ci co o", ci=P))
                w_sb[name] = t

            mb_sb = singles.tile([P, KT_TILES], F32, tag="mb")
            nc.sync.dma_start(mb_sb[:], maskbias.rearrange("(kt ki) -> ki kt", ki=P))

            if qkv_bias:
                bq_sb = singles.tile([P, CO], F32, tag="bq")
                nc.sync.dma_start(bq_sb[:], bq.rearrange("(co ci) -> ci co", ci=P))
                bk_sb = singles.tile([P, CO], F32, tag="bk")
                nc.sync.dma_start(bk_sb[:], bk.rearrange("(co ci) -> ci co", ci=P))
                bv_bc = singles.tile([P, D_MODEL], F32, tag="bv")
                nc.sync.dma_start(bv_bc[:], bv.to_broadcast((P, D_MODEL)))
            if ln_affine:
                gamma_bc = singles.tile([P, D_MODEL], F32, tag="gamma")
                nc.sync.dma_start(gamma_bc[:], gamma.to_broadcast((P, D_MODEL)))
                beta_bc = singles.tile([P, D_MODEL], F32, tag="beta")
                nc.sync.dma_start(beta_bc[:], beta.to_broadcast((P, D_MODEL)))
            eps_sb = singles.tile([P, 1], F32, tag="eps")
            nc.gpsimd.memset(eps_sb[:], 1e-5)
            ones128 = singles.tile([P, 1], BF16, tag="ones128")
            nc.gpsimd.memset(ones128[:], 1.0)
            ones1 = singles.tile([1, D_K], BF16, tag="ones1")
            nc.gpsimd.memset(ones1[:], 1.0)

            # ---- load activations (transposed layouts) ---------------------
            qT_sb = inbuf.tile([P, CO, SQ], BF16, tag="qT")
            nc.sync.dma_start(qT_sb[:], qT.rearrange("(co ci) q -> ci co q", ci=P))
            kT_sb = inbuf.tile([P, CO, SK], BF16, tag="kT")
            nc.sync.dma_start(kT_sb[:], kT.rearrange("(co ci) k -> ci co k", ci=P))
            vT_sb = inbuf.tile([P, CO, SK], BF16, tag="vT")
            nc.sync.dma_start(vT_sb[:], vT.rearrange("(co ci) k -> ci co k", ci=P))

            # ---- projections (psums ride in the "sc" slots) ----------------
            QT_sb = singles.tile([P, CO, SQ], BF16, tag="QT")
            for do in range(CO):
                for q0 in range(0, SQ, 512):
                    psq = ps_sc.tile([P, 512], F32, tag="sc")
                    for ci in range(CO):
                        nc.tensor.matmul(
                            psq[:], w_sb["wq"][:, ci, do * P:(do + 1) * P],
                            qT_sb[:, ci, q0:q0 + 512],
                            start=(ci == 0), stop=(ci == CO - 1))
                    dst = QT_sb[:, do, q0:q0 + 512]
                    if qkv_bias:
                        nc.vector.tensor_scalar_add(dst, psq[:], bq_sb[:, do:do + 1])
                    else:
                        nc.vector.tensor_copy(dst, psq[:])

            KT_sb = singles.tile([P, CO, SK], BF16, tag="KT")
            for do in range(CO):
                for k0 in range(0, SK, 512):
                    psk = ps_sc.tile([P, 512], F32, tag="sc")
                    for ci in range(CO):
                        nc.tensor.matmul(
                            psk[:], w_sb["wk"][:, ci, do * P:(do + 1) * P],
                            kT_sb[:, ci, k0:k0 + 512],
                            start=(ci == 0), stop=(ci == CO - 1))
                    dst = KT_sb[:, do, k0:k0 + 512]
                    if qkv_bias:
                        nc.vector.tensor_scalar_add(dst, psk[:], bk_sb[:, do:do + 1])
                    else:
                        nc.vector.tensor_copy(dst, psk[:])

            V_sb = singles.tile([P, KT_TILES, N_HEADS, D_K], BF16, tag="V")
            for st in range(KT_TILES):
                psv = ps_sc.tile([P, 512], F32, tag="sc")
                for ci in range(CO):
                    nc.tensor.matmul(
                        psv[:], vT_sb[:, ci, st * P:(st + 1) * P],
                        w_sb["wv"][:, ci, :],
                        start=(ci == 0), stop=(ci == CO - 1))
                dst = V_sb[:, st, :, :]
                src = psv[:].rearrange("p (h e) -> p h e", h=N_HEADS)
                if qkv_bias:
                    nc.vector.tensor_tensor(
                        dst, src,
                        bv_bc[:].rearrange("p (h e) -> p h e", h=N_HEADS),
                        ALU.add)
                else:
                    nc.vector.tensor_copy(dst, src)

            # ---- attention: head pairs, full 1024-q tiles ------------------
            # ctx_sb[d, do, q] pair-major (matches O-proj lhsT layout)
            ctx_sb = singles.tile([P, CO, SQ], BF16, tag="ctx")
            # softmax denominators, row j = (head, q-chunk) = 2h + c
            sums_sb = singles.tile([2 * N_HEADS, 512], F32, tag="sums")
            sums_stage = {}  # (group) -> strided psum tile with 4 sums rows

            for pair in range(NPAIR):
                hA, hB = 2 * pair, 2 * pair + 1
                do = pair
                grp, gi = divmod(pair, 2)
                if gi == 0:
                    sums_stage[grp] = ps_sums.tile([P, 2, 512], F32, tag="sums", name=f"sums_g{grp}")
                sm = sums_stage[grp]
                rA, rB = 64 * gi, 64 * gi + 32
                ctx_ps = ps_ctx.tile([P, 2, 512], F32, tag="ctx")

                def consume(kt, ptA, ptB):
                    # ctx/sums matmuls for tile kt (PT already computed)
                    first, last = kt == 0, kt == KT_TILES - 1
                    for c in range(2):
                        q0 = c * 512
                        nc.tensor.matmul(
                            ctx_ps[0:D_K, c, :], V_sb[:, kt, hA, :],
                            ptA[:, q0:q0 + 512], start=first, stop=last,
                            tile_position=(0, 0), skip_group_check=True)
                        nc.tensor.matmul(
                            ctx_ps[D_K:P, c, :], V_sb[:, kt, hB, :],
                            ptB[:, q0:q0 + 512], start=first, stop=last,
                            tile_position=(0, D_K), skip_group_check=True)
                        nc.tensor.matmul(
                            sm[rA:rA + 1, c, :], ones128[:],
                            ptA[:, q0:q0 + 512], start=first, stop=last,
                            tile_position=(0, rA), skip_group_check=True)
                        nc.tensor.matmul(
                            sm[rB:rB + 1, c, :], ones128[:],
                            ptB[:, q0:q0 + 512], start=first, stop=last,
                            tile_position=(0, rB), skip_group_check=True)

                pending = None  # 1-tile software pipeline: PE consumes PT
                for kt in range(KT_TILES):  # while ACT works on the next one
                    k0 = kt * P
                    sA = ps_sc.tile([P, 1024], F32, tag="sc")
                    sB = ps_sc.tile([P, 1024], F32, tag="sc")
                    for c in range(2):
                        q0 = c * 512
                        nc.tensor.matmul(
                            sA[:, q0:q0 + 512],
                            KT_sb[0:D_K, do, k0:k0 + P],
                            QT_sb[0:D_K, do, q0:q0 + 512],
                            start=True, stop=True)
                        nc.tensor.matmul(
                            sB[:, q0:q0 + 512],
                            KT_sb[D_K:P, do, k0:k0 + P],
                            QT_sb[D_K:P, do, q0:q0 + 512],
                            start=True, stop=True)
                    ptA = ptp.tile([P, 1024], BF16, tag="pt")
                    nc.scalar.activation(ptA[:], sA[:], AF.Exp,
                                         bias=mb_sb[:, kt:kt + 1], scale=0.125)
                    ptB = ptp.tile([P, 1024], BF16, tag="pt")
                    nc.scalar.activation(ptB[:], sB[:], AF.Exp,
                                         bias=mb_sb[:, kt:kt + 1], scale=0.125)
                    if pending is not None:
                        consume(*pending)
                    pending = (kt, ptA, ptB)
                consume(*pending)
                # unnormalized context -> SBUF bf16 (normalized later)
                nc.vector.tensor_copy(
                    ctx_sb[:, do, :].rearrange("p (c q) -> p c q", c=2),
                    ctx_ps[:])
                if gi == 1:
                    # both pairs of this group done: move the 4 sums rows
                    # (partitions 0,32,64,96) out of PSUM, then to their
                    # (head, chunk) slots in sums_sb via DMA partition moves
                    sst = small.tile([P, 2, 512], F32, tag="sst")
                    for r in (0, 32, 64, 96):
                        nc.vector.tensor_copy(sst[r:r + 1, :, :],
                                              sm[r:r + 1, :, :])
                    for i, h in enumerate(range(2 * (pair - 1), 2 * pair + 2)):
                        r = 32 * i
                        for c in range(2):
                            nc.sync.dma_start(sums_sb[2 * h + c:2 * h + c + 1, :],
                                              sst[r:r + 1, c, :])

            # ---- normalize context ----------------------------------------
            recip = singles.tile([2 * N_HEADS, 512], F32, tag="recip")
            nc.vector.reciprocal(recip[:], sums_sb[:])
            recip_bf = singles.tile([2 * N_HEADS, 512], BF16, tag="recipbf")
            nc.vector.tensor_copy(recip_bf[:], recip[:])
            # stage rows into partition 0's free dim, then broadcast each
            # across 64 PSUM partitions via 1-row PE outer products
            stage = singles.tile([1, 2 * N_HEADS, 512], BF16, tag="stg")
            for j in range(2 * N_HEADS):
                nc.sync.dma_start(stage[:, j, :], recip_bf[j:j + 1, :])
            for pair in range(NPAIR):
                hA, hB = 2 * pair, 2 * pair + 1
                for c in range(2):
                    rb = ps_sc.tile([P, 512], F32, tag="sc")
                    nc.tensor.matmul(rb[0:D_K, :], ones1[:],
                                     stage[:, 2 * hA + c, :],
                                     start=True, stop=True,
                                     tile_position=(0, 0),
                                     skip_group_check=True)
                    nc.tensor.matmul(rb[D_K:P, :], ones1[:],
                                     stage[:, 2 * hB + c, :],
                                     start=True, stop=True,
                                     tile_position=(0, D_K),
                                     skip_group_check=True)
                    cslice = ctx_sb[:, pair, c * 512:(c + 1) * 512]
                    nc.vector.tensor_tensor(cslice, cslice, rb[:], ALU.mult)

            # ---- output projection + residual + layernorm ------------------
            for qc in range(SQ // P):
                zps = ps_sc.tile([P, D_MODEL], F32, tag="sc")
                for do in range(CO):
                    nc.tensor.matmul(
                        zps[:], ctx_sb[:, do, qc * P:(qc + 1) * P],
                        w_sb["wo"][:, do, :],
                        start=(do == 0), stop=(do == CO - 1))
                qr = stream.tile([P, D_MODEL], F32, tag="qr")
                nc.sync.dma_start(qr[:], qres[qc * P:(qc + 1) * P, :])
                z = stream.tile([P, D_MODEL], F32, tag="z")
                nc.vector.tensor_tensor(z[:], zps[:], qr[:], ALU.add)
                stats = small.tile([P, 6], F32, tag="stats")
                nc.vector.bn_stats(stats[:], z[:])
                mv = small.tile([P, 2], F32, tag="mv")
                nc.vector.bn_aggr(mv[:], stats[:])
                istd = small.tile([P, 1], F32, tag="istd")
                nc.scalar.activation(istd[:], mv[:, 1:2], AF.Sqrt,
                                     bias=eps_sb[:], scale=1.0)
                nc.vector.reciprocal(istd[:], istd[:])
                zo = stream.tile([P, D_MODEL], F32, tag="zo")
                nc.vector.tensor_scalar(zo[:], z[:], mv[:, 0:1], istd[:],
                                        ALU.subtract, ALU.mult)
                if ln_affine:
                    nc.vector.tensor_tensor(zo[:], zo[:], gamma_bc[:], ALU.mult)
                    nc.vector.tensor_tensor(zo[:], zo[:], beta_bc[:], ALU.add)
                nc.sync.dma_start(out[qc * P:(qc + 1) * P, :], zo[:])

    nc.compile()
    return nc


def _get_nc(qkv_bias: bool, ln_affine: bool):
    key = (qkv_bias, ln_affine)
    if key not in _NC_CACHE:
        _NC_CACHE[key] = _build(*key)
    return _NC_CACHE[key]


def prepare(query, key, value, key_mask, Wq, bq, Wk, bk, Wv, bv, Wo, bo,
            ln_gamma, ln_beta):
    """Host-side prep: returns (nc, in_maps) for the 8 cores."""
    query = np.asarray(query, dtype=np.float32)
    key = np.asarray(key, dtype=np.float32)
    value = np.asarray(value, dtype=np.float32)
    key_mask = np.asarray(key_mask)
    Wq = np.asarray(Wq, dtype=np.float32)
    Wk = np.asarray(Wk, dtype=np.float32)
    Wv = np.asarray(Wv, dtype=np.float32)
    Wo = np.asarray(Wo, dtype=np.float32)
    bq = np.asarray(bq, dtype=np.float32)
    bk = np.asarray(bk, dtype=np.float32)
    bv = np.asarray(bv, dtype=np.float32)
    bo = np.asarray(bo, dtype=np.float32)
    ln_gamma = np.asarray(ln_gamma, dtype=np.float32)
    ln_beta = np.asarray(ln_beta, dtype=np.float32)

    B, sq_full, dm = query.shape
    assert (B, sq_full, dm) == (4, 2048, 512), query.shape

    qkv_bias = bool(bq.any() or bk.any() or bv.any())
    ln_affine = bool((ln_gamma != 1.0).any() or ln_beta.any())
    nc = _get_nc(qkv_bias, ln_affine)

    bf = ml_dtypes.bfloat16
    wqT = np.ascontiguousarray(Wq.T).astype(bf)
    wkT = np.ascontiguousarray(Wk.T).astype(bf)
    wvT = np.ascontiguousarray(Wv.T).astype(bf)
    woT = np.ascontiguousarray(Wo.T).astype(bf)
    maskbias = np.where(key_mask, np.float32(0.0), np.float32(NEG))
    qres_full = query + bo[None, None, :]

    in_maps = []
    for core in range(N_CORES):
        b, half = divmod(core, 2)
        rows = slice(half * SQ, (half + 1) * SQ)
        m = {
            "qT": np.ascontiguousarray(query[b, rows].T).astype(bf),
            "kT": np.ascontiguousarray(key[b].T).astype(bf),
            "vT": np.ascontiguousarray(value[b].T).astype(bf),
            "qres": np.ascontiguousarray(qres_full[b, rows]),
            "wqT": wqT, "wkT": wkT, "wvT": wvT, "woT": woT,
            "maskbias": np.ascontiguousarray(maskbias[b]),
        }
        if qkv_bias:
            m["bq"], m["bk"], m["bv"] = bq, bk, bv
        if ln_affine:
            m["gamma"], m["beta"] = ln_gamma, ln_beta
        in_maps.append(m)
    return nc, in_maps


def kernel(**inputs):
    nc, in_maps = prepare(**inputs)
    B, sq_full, dm = 4, 2048, 512

    res = bass_utils.run_bass_kernel_spmd(nc, in_maps,
                                          core_ids=list(range(N_CORES)))
    out = np.empty((B, sq_full, dm), dtype=np.float32)
    for core in range(N_CORES):
        b, half = divmod(core, 2)
        out[b, half * SQ:(half + 1) * SQ] = res.results[core]["out"]
    return out
